# revision 4
# baseline (speedup 1.0000x reference)
"""GraphConv GNN kernel for trn2: host preprocessing + bass program builder.

Sharding: nodes (and incident edges, by dst) across 8 cores. Weights
replicated. Structural optimizations over the dma_gather-everywhere baseline
(which was GpSimd-bound on gather descriptor generation):

- Layer 1: the gather of x[src] is a host-side permutation of an input
  tensor, so it is pre-gathered on the host into a sequential stream and
  DMA'd in chunk order (no dma_gather, no Q7 work).
- Layer 2: real dma_gather from the AllGather'd h1 (unavoidable: h1 is
  computed on device and relu is nonlinear). Sources are split into region
  A/B by the source node's local index half, and h1 is exchanged with TWO
  AllGathers (A then B) so the region-A gathers start while the region-B
  AllGather is still in flight.
- Layer 3 is linear and feeds only mean-pooling, so pooling commutes with
  it: pool(agg3)[g] = sum_u C[u,g] h2[u] with C[u,g] = #edges from node u
  into graph g (host-built count matrix), and pool(h2) uses the batch
  one-hot. Both are small dense matmuls over local node chunks; partial
  sums are combined on the host. This removes the layer-3 gather, its
  one-hot streams, and the second h-AllGather entirely.
"""

import sys

sys.path.insert(0, "/opt/trn_rl_repo")

import numpy as np
import ml_dtypes

import concourse.bass as bass
import concourse.bacc as bacc
import concourse.tile as tile
import concourse.mybir as mybir
from concourse import library_config

BF16 = mybir.dt.bfloat16
F32 = mybir.dt.float32
I16 = mybir.dt.int16

N_CORES = 8
F = 128
N_CLASSES = 10

# per-window structure: K_A region-A chunks + K_B region-B chunks of 128 edges
K_A = 6
K_B = 6
EDGES_PER_HALF = K_A * 128  # 768
CHUNKS_PER_WIN = K_A + K_B
CPO = 32  # gather chunks per dma_gather op (4096 idxs)
S_FP8 = True  # layer-1 one-hot streams stored fp8, cast to bf16 on DMA


def _wrap_idx(idx_flat):
    """idx i -> partition i%16, col i//16; replicated across the 8 Q7 core
    stripes (16 partitions each)."""
    n = idx_flat.shape[0]
    return np.ascontiguousarray(
        np.tile(idx_flat.reshape(n // 16, 16).T.astype(np.int16), (8, 1))
    )


def preprocess(x, edge_index, batch, params, n_nodes, n_graphs):
    """Build per-core inputs + meta for the SPMD program."""
    assert n_nodes % N_CORES == 0
    npc = n_nodes // N_CORES
    halfn = npc // 2
    src = np.asarray(edge_index[0], np.int64)
    dst = np.asarray(edge_index[1], np.int64)
    batch = np.asarray(batch, np.int64)
    x = np.asarray(x, np.float32)

    # region A: source's LOCAL index (within its owner core) < halfn
    # sort edges by dst once
    order = np.argsort(dst, kind="stable")
    src_s, dst_s = src[order], dst[order]

    # per-core edge ranges
    core_edge_start = np.searchsorted(dst_s, np.arange(0, n_nodes + 1, npc))

    # --- pass 1: greedy windows per core (forced break at halfn) ---
    core_windows = []  # per core: list of (dst_start, dst_end) local
    core_wA = []  # windows covering dst < halfn
    for k in range(N_CORES):
        e0, e1 = core_edge_start[k], core_edge_start[k + 1]
        dl = dst_s[e0:e1] - k * npc
        sl_a = (src_s[e0:e1] % npc) < halfn
        deg_a = np.bincount(dl[sl_a], minlength=npc)
        deg_b = np.bincount(dl[~sl_a], minlength=npc)
        wins = []
        d = 0
        while d < npc:
            start = d
            brk = halfn if d < halfn else npc
            a_c = b_c = 0
            while (
                d < brk
                and d - start < 128
                and a_c + deg_a[d] <= EDGES_PER_HALF
                and b_c + deg_b[d] <= EDGES_PER_HALF
            ):
                a_c += deg_a[d]
                b_c += deg_b[d]
                d += 1
            assert d > start, "single dst exceeds per-window edge budget"
            wins.append((start, d))
        core_windows.append(wins)
        core_wA.append(sum(1 for a, _ in wins if a < halfn))

    wA_star = max(core_wA)
    wB_star = max(len(w) - a for w, a in zip(core_windows, core_wA))
    w_star = wA_star + wB_star
    if w_star % 4:  # keep ls a multiple of 512
        wB_star += 4 - (w_star % 4)
        w_star = wA_star + wB_star
    ls = w_star * 128
    lsA, lsB = wA_star * 128, wB_star * 128
    rowsA, rowsB = N_CORES * lsA, N_CORES * lsB
    assert max(rowsA, rowsB) <= 32768, f"{rowsA=} {rowsB=} exceed int16 idx range"

    # --- slots for every node (A windows at 0..wA*-1, B at wA*..w*-1) ---
    slot = np.full(n_nodes, -1, np.int64)
    for k in range(N_CORES):
        wA_k = core_wA[k]
        for w, (a, b) in enumerate(core_windows[k]):
            w_slab = w if w < wA_k else wA_star + (w - wA_k)
            d_loc = np.arange(a, b)
            slot[k * npc + d_loc] = w_slab * 128 + (d_loc - a)
    assert (slot >= 0).all()
    owner = np.arange(n_nodes) // npc
    wslab = slot // 128
    in_A = wslab < wA_star
    # region-local row index (fm pos s -> (p=s%128, c=s//128); partition-major
    # DRAM -> row = p*W + c)
    row_reg = np.where(
        in_A,
        owner * lsA + (slot % 128) * wA_star + wslab,
        owner * lsB + (slot % 128) * wB_star + (wslab - wA_star),
    )
    # sanity: A-region nodes are exactly the first-half locals
    assert (in_A == ((np.arange(n_nodes) % npc) < halfn)).all()

    x_bf = x.astype(ml_dtypes.bfloat16)

    # --- per-core streams (keys: "a" region A sources, "b" region B) ---
    per_core = []
    for k in range(N_CORES):
        e0, e1 = core_edge_start[k], core_edge_start[k + 1]
        dl = dst_s[e0:e1] - k * npc
        sv = src_s[e0:e1]
        is_a = (sv % npc) < halfn
        idx_a = np.zeros((w_star, EDGES_PER_HALF), np.int64)
        ids_a = np.full((w_star, EDGES_PER_HALF), -1.0, np.float32)
        src_a = np.full((w_star, EDGES_PER_HALF), -1, np.int64)
        idx_b = np.zeros_like(idx_a)
        ids_b = np.full_like(ids_a, -1.0)
        src_b = np.full_like(src_a, -1)
        wbounds = np.searchsorted(
            dl, [a for a, _ in core_windows[k]] + [npc]
        )
        wA_k = core_wA[k]
        for w, (a, b) in enumerate(core_windows[k]):
            w_slab = w if w < wA_k else wA_star + (w - wA_k)
            a_m = is_a[wbounds[w] : wbounds[w + 1]]
            e_dst = dl[wbounds[w] : wbounds[w + 1]]
            e_src = sv[wbounds[w] : wbounds[w + 1]]
            for half, m in ((0, a_m), (1, ~a_m)):
                r = row_reg[e_src[m]]
                cnt = r.shape[0]
                assert cnt <= EDGES_PER_HALF
                tgt_idx = idx_a if half == 0 else idx_b
                tgt_ids = ids_a if half == 0 else ids_b
                tgt_src = src_a if half == 0 else src_b
                tgt_idx[w_slab, :cnt] = r
                tgt_ids[w_slab, :cnt] = (e_dst[m] - a).astype(np.float32)
                tgt_src[w_slab, :cnt] = e_src[m]

        def _onehot(ids_arr, dt):
            nch = ids_arr.size // 128
            ids_r = ids_arr.reshape(nch, 128)
            oh = (ids_r[:, :, None] == np.arange(128, dtype=np.float32)[None, None, :])
            return np.ascontiguousarray(
                oh.transpose(1, 0, 2).reshape(128, nch * 128).astype(dt))

        def _pregather(src_arr):
            # slot j (chunk c=j//128, p=j%128) -> x[src]; layout [128, nch*F]
            flat = src_arr.reshape(-1)
            g = np.zeros((flat.shape[0], F), ml_dtypes.bfloat16)
            v = flat >= 0
            g[v] = x_bf[flat[v]]
            nch = flat.shape[0] // 128
            return np.ascontiguousarray(
                g.reshape(nch, 128, F).transpose(1, 0, 2).reshape(128, nch * F))

        sdt = ml_dtypes.float8_e4m3fn if S_FP8 else ml_dtypes.bfloat16
        per_core.append(
            dict(
                idx_a=_wrap_idx(idx_a.reshape(-1)),
                idx_b=_wrap_idx(idx_b.reshape(-1)),
                s_a=_onehot(ids_a.reshape(-1), ml_dtypes.bfloat16),
                s_b=_onehot(ids_b.reshape(-1), ml_dtypes.bfloat16),
                s8_a=_onehot(ids_a.reshape(-1), sdt),
                s8_b=_onehot(ids_b.reshape(-1), sdt),
                xg_a=_pregather(src_a),
                xg_b=_pregather(src_b),
            )
        )

    # --- per-(src node, graph) edge-count matrix for the pooled layer-3 ---
    gmax = 64
    c_full = np.zeros((n_nodes, gmax), np.float32)
    np.add.at(c_full, (src, batch[dst]), 1.0)

    def _node_major_64(vals_per_node, k):
        """vals [npc, 64] for core k's local nodes -> [128, w_star*64] in
        node-major chunk layout (row p, block c) = node at slot c*128+p."""
        out = np.zeros((ls, gmax), np.float32)
        g = np.arange(k * npc, (k + 1) * npc)
        out[slot[g]] = vals_per_node
        out = out.reshape(w_star, 128, gmax).transpose(1, 0, 2)
        return np.ascontiguousarray(
            out.reshape(128, w_star * gmax).astype(ml_dtypes.bfloat16))

    in_maps = []
    for k in range(N_CORES):
        g = np.arange(k * npc, (k + 1) * npc)
        x_fm = np.zeros((F, ls), ml_dtypes.bfloat16)
        x_fm[:, slot[g]] = x_bf[g].T
        b_vals = np.zeros((npc, gmax), np.float32)
        b_vals[np.arange(npc), batch[g]] = 1.0
        m = dict(
            x_fm=x_fm,
            b_onehot=_node_major_64(b_vals, k),
            c_onehot=_node_major_64(c_full[g], k),
            idx_a=per_core[k]["idx_a"],
            idx_b=per_core[k]["idx_b"],
            s_a=per_core[k]["s_a"],
            s_b=per_core[k]["s_b"],
            xg_a=per_core[k]["xg_a"],
            xg_b=per_core[k]["xg_b"],
            w1relT=np.ascontiguousarray(params["W1_rel"].T.astype(ml_dtypes.bfloat16)),
            w1rootT=np.ascontiguousarray(
                params["W1_root"].T.astype(ml_dtypes.bfloat16)
            ),
            w2relT=np.ascontiguousarray(params["W2_rel"].T.astype(ml_dtypes.bfloat16)),
            w2rootT=np.ascontiguousarray(
                params["W2_root"].T.astype(ml_dtypes.bfloat16)
            ),
            w3relT=np.ascontiguousarray(params["W3_rel"].T.astype(np.float32)),
            w3rootT=np.ascontiguousarray(
                params["W3_root"].T.astype(np.float32)
            ),
            b1=np.ascontiguousarray(params["b1_rel"].astype(np.float32).reshape(F, 1)),
            b2=np.ascontiguousarray(params["b2_rel"].astype(np.float32).reshape(F, 1)),
            wlinT=np.ascontiguousarray(params["W_lin"].T.astype(np.float32)),
        )
        if S_FP8:
            m["s8_a"] = per_core[k]["s8_a"]
            m["s8_b"] = per_core[k]["s8_b"]
        in_maps.append(m)

    meta = dict(w_star=w_star, wA_star=wA_star, wB_star=wB_star,
                ls=ls, lsA=lsA, lsB=lsB, rowsA=rowsA, rowsB=rowsB,
                n_graphs=n_graphs)
    return meta, in_maps


def build_nc(meta, n_graphs_pad=64):
    w_star = meta["w_star"]
    wA_star, wB_star = meta["wA_star"], meta["wB_star"]
    ls, lsA, lsB = meta["ls"], meta["lsA"], meta["lsB"]
    rowsA, rowsB = meta["rowsA"], meta["rowsB"]
    sl_len = w_star * EDGES_PER_HALF  # idxs per region stream
    n_chunks = sl_len // 128
    dw = ls // 512  # dense windows
    ng = n_graphs_pad
    FP8 = mybir.dt.float8e4

    nc = bacc.Bacc(
        "TRN2", target_bir_lowering=False, debug=False, num_devices=N_CORES
    )

    # --- I/O ---
    x_fm_d = nc.dram_tensor("x_fm", [F, ls], BF16, kind="ExternalInput")
    bone_d = nc.dram_tensor("b_onehot", [128, w_star * 64], BF16, kind="ExternalInput")
    cone_d = nc.dram_tensor("c_onehot", [128, w_star * 64], BF16, kind="ExternalInput")
    idx_d = {
        "a": nc.dram_tensor("idx_a", [128, sl_len // 16], I16, kind="ExternalInput"),
        "b": nc.dram_tensor("idx_b", [128, sl_len // 16], I16, kind="ExternalInput"),
    }
    s_d = {
        "a": nc.dram_tensor("s_a", [128, n_chunks * 128], BF16, kind="ExternalInput"),
        "b": nc.dram_tensor("s_b", [128, n_chunks * 128], BF16, kind="ExternalInput"),
    }
    if S_FP8:
        s8_d = {
            "a": nc.dram_tensor("s8_a", [128, n_chunks * 128], FP8, kind="ExternalInput"),
            "b": nc.dram_tensor("s8_b", [128, n_chunks * 128], FP8, kind="ExternalInput"),
        }
    xg_d = {
        "a": nc.dram_tensor("xg_a", [128, n_chunks * F], BF16, kind="ExternalInput"),
        "b": nc.dram_tensor("xg_b", [128, n_chunks * F], BF16, kind="ExternalInput"),
    }
    w_d = {}
    for l in (1, 2):
        for p in ("rel", "root"):
            w_d[l, p] = nc.dram_tensor(f"w{l}{p}T", [F, F], BF16, kind="ExternalInput")
    w3_d = {
        "rel": nc.dram_tensor("w3relT", [F, F], F32, kind="ExternalInput"),
        "root": nc.dram_tensor("w3rootT", [F, F], F32, kind="ExternalInput"),
    }
    b_d = {l: nc.dram_tensor(f"b{l}", [F, 1], F32, kind="ExternalInput") for l in (1, 2)}
    wlin_d = nc.dram_tensor("wlinT", [F, N_CLASSES], F32, kind="ExternalInput")
    out_d = nc.dram_tensor("out_partial", [N_CLASSES, ng], F32, kind="ExternalOutput")

    relu = mybir.ActivationFunctionType.Relu
    copy_f = mybir.ActivationFunctionType.Copy

    n_ops = (n_chunks + CPO - 1) // CPO

    with tile.TileContext(nc) as tc:
        with (
            tc.tile_pool(name="const", bufs=1) as constp,
            tc.tile_pool(name="state", bufs=1) as statep,
            tc.tile_pool(name="gpool", bufs=2) as gpool,
            tc.tile_pool(name="spool", bufs=8) as spool,
            tc.tile_pool(name="psa", bufs=2, space="PSUM") as psa,
            tc.tile_pool(name="psd", bufs=2, space="PSUM") as psd,
            tc.tile_pool(name="psp", bufs=1, space="PSUM") as psp,
            tc.tile_pool(name="dram", bufs=1, space="DRAM") as dramp,
        ):
            nc.gpsimd.load_library(library_config.mlp)

            # ---- load constants ----
            bone_t = constp.tile([128, w_star * 64], BF16)
            nc.sync.dma_start(bone_t[:], bone_d[:])
            cone_t = constp.tile([128, w_star * 64], BF16)
            nc.sync.dma_start(cone_t[:], cone_d[:])
            idx_t = {}
            for h in ("a", "b"):
                it = constp.tile([128, sl_len // 16], I16, name=f"idx_{h}")
                nc.sync.dma_start(it[:], idx_d[h][:])
                idx_t[h] = it
            w_t = {}
            for key, d in w_d.items():
                wt = constp.tile([F, F], BF16, name=f"w_{key[0]}_{key[1]}")
                nc.sync.dma_start(wt[:], d[:])
                w_t[key] = wt
            w3_t = {}
            for p, d in w3_d.items():
                wt = constp.tile([F, F], F32, name=f"w3_{p}")
                nc.sync.dma_start(wt[:], d[:])
                w3_t[p] = wt
            b_t = {}
            for l, d in b_d.items():
                bt = constp.tile([F, 1], F32, name=f"b_{l}")
                nc.sync.dma_start(bt[:], d[:])
                b_t[l] = bt
            wlin_t = constp.tile([F, N_CLASSES], F32)
            nc.sync.dma_start(wlin_t[:], wlin_d[:])

            x_fm_t = statep.tile([F, ls], BF16, tag="h0")
            nc.sync.dma_start(x_fm_t[:], x_fm_d[:])

            # ---- layers 1 (pre-gathered stream) and 2 (dma_gather) ----
            h_fm = x_fm_t
            hf = {}  # layer-2 gather sources (set after AllGathers)
            for layer in (1, 2):
                g_tiles = {"a": [], "b": []}
                s_tiles = {"a": [], "b": []}
                # region A ops first, then region B (B AllGather overlaps A ops)
                for h in ("a", "b"):
                    for o in range(n_ops):
                        c0 = o * CPO
                        c1 = min(n_chunks, c0 + CPO)
                        nch = c1 - c0
                        st_ = spool.tile(
                            [128, nch * 128],
                            BF16,
                            name=f"sg_{layer}_{h}_{o}",
                            tag=f"sg_{h}",
                            bufs=2,
                        )
                        if layer == 1 and S_FP8:
                            nc.gpsimd.dma_start(
                                st_[:], s8_d[h][:, c0 * 128 : c1 * 128]
                            )
                        else:
                            nc.sync.dma_start(
                                st_[:], s_d[h][:, c0 * 128 : c1 * 128]
                            )
                        s_tiles[h].append(st_)
                        gt = gpool.tile(
                            [128, nch, F],
                            BF16,
                            name=f"g_{layer}_{h}_{o}",
                            tag=f"g_{h}",
                            padded_shape=[128, CPO, F],
                        )
                        if layer == 1:
                            nc.sync.dma_start(
                                gt[:], xg_d[h][:, c0 * F : c1 * F]
                            )
                        else:
                            nidx = nch * 128
                            nc.gpsimd.dma_gather(
                                gt[:],
                                hf[h][:],
                                idx_t[h][:, c0 * 8 : c1 * 8],
                                nidx,
                                nidx,
                                F,
                                single_packet=False,
                            )
                        g_tiles[h].append(gt)

                # aggregation windows
                agg_fm = statep.tile([F, ls], BF16, tag=f"agg{layer}", name=f"agg_{layer}")
                for w in range(w_star):
                    ps = psa.tile([128, 128], F32, name=f"psagg_{layer}_{w}", tag="psagg")
                    for j in range(CHUNKS_PER_WIN):
                        h = "a" if j < K_A else "b"
                        cc = w * K_A + (j % K_A)
                        o, sl_ = cc // CPO, cc % CPO
                        nc.tensor.matmul(
                            ps[:],
                            g_tiles[h][o][:, sl_, :],
                            s_tiles[h][o][:, sl_ * 128 : (sl_ + 1) * 128],
                            start=(j == 0),
                            stop=(j == CHUNKS_PER_WIN - 1),
                        )
                    nc.scalar.activation(
                        agg_fm[:, w * 128 : (w + 1) * 128], ps[:], copy_f
                    )

                # dense
                h_next = statep.tile([F, ls], BF16, tag=f"h{layer}", name=f"h_{layer}")
                for d in range(dw):
                    ps = psd.tile([128, 512], F32, name=f"psd_{layer}_{d}", tag="psd")
                    sl2 = slice(d * 512, (d + 1) * 512)
                    nc.tensor.matmul(
                        ps[:], w_t[layer, "rel"][:], agg_fm[:, sl2], start=True, stop=False
                    )
                    nc.tensor.matmul(
                        ps[:], w_t[layer, "root"][:], h_fm[:, sl2], start=False, stop=True
                    )
                    nc.scalar.activation(
                        h_next[:, sl2],
                        ps[:],
                        relu,
                        bias=b_t[layer][:],
                    )

                if layer == 1:
                    # share h1 for the layer-2 gather: region A first so AG_A
                    # lands while AG_B is still in flight.
                    h_nm = statep.tile(
                        [128, w_star, F], BF16, tag="h1nm", name="hnm_1"
                    )
                    for h, w0, w1, lsR, rowsR in (
                        ("a", 0, wA_star, lsA, rowsA),
                        ("b", wA_star, w_star, lsB, rowsB),
                    ):
                        nc.sync.dma_start_transpose(
                            h_nm[:, w0:w1, :], h_next[:, w0 * 128 : w1 * 128]
                        )
                        ag_in = dramp.tile(
                            [128, lsR], BF16, name=f"agin_{h}", tag=f"agin{h}"
                        )
                        hf_r = dramp.tile(
                            [rowsR, F],
                            BF16,
                            name=f"hf_{h}",
                            tag=f"hf{h}",
                            addr_space="Shared",
                        )
                        nc.sync.dma_start(ag_in[:], h_nm[:, w0:w1, :])
                        nc.gpsimd.collective_compute(
                            "AllGather",
                            mybir.AluOpType.bypass,
                            replica_groups=[list(range(N_CORES))],
                            ins=[ag_in[:]],
                            outs=[hf_r[:]],
                        )
                        hf[h] = hf_r
                    h_fm = h_next
                else:
                    h2_fm = h_next

            # ---- pooled layer 3: pool(agg3) = C^T h2, pool(h2) = B^T h2 ----
            h2_nm = statep.tile([128, w_star, F], BF16, tag="h2nm", name="hnm_2")
            nc.sync.dma_start_transpose(h2_nm[:], h2_fm[:])

            ps_pa = psp.tile([128, ng], F32, tag="pspa")
            ps_ph = psp.tile([128, ng], F32, tag="psph")
            for c in range(w_star):
                nc.tensor.matmul(
                    ps_pa[:],
                    h2_nm[:, c, :],
                    cone_t[:, c * ng : (c + 1) * ng],
                    start=(c == 0),
                    stop=(c == w_star - 1),
                )
            for c in range(w_star):
                nc.tensor.matmul(
                    ps_ph[:],
                    h2_nm[:, c, :],
                    bone_t[:, c * ng : (c + 1) * ng],
                    start=(c == 0),
                    stop=(c == w_star - 1),
                )
            pa_sb = statep.tile([128, ng], F32, tag="pasb")
            ph_sb = statep.tile([128, ng], F32, tag="phsb")
            nc.scalar.activation(pa_sb[:], ps_pa[:], copy_f)
            nc.scalar.activation(ph_sb[:], ps_ph[:], copy_f)

            # z3p = W3_rel @ pool(agg3) + W3_root @ pool(h2)   [128, ng] f32
            ps_z = psp.tile([128, ng], F32, tag="psz")
            nc.tensor.matmul(ps_z[:], w3_t["rel"][:], pa_sb[:], start=True, stop=False)
            nc.tensor.matmul(ps_z[:], w3_t["root"][:], ph_sb[:], start=False, stop=True)
            z_sb = statep.tile([128, ng], F32, tag="zsb")
            nc.vector.tensor_copy(z_sb[:], ps_z[:])

            ps_head = psp.tile([N_CLASSES, ng], F32, tag="pshead")
            nc.tensor.matmul(ps_head[:], wlin_t[:], z_sb[:])
            out_sb = statep.tile([N_CLASSES, ng], F32, tag="outsb")
            nc.vector.tensor_copy(out_sb[:], ps_head[:])
            nc.sync.dma_start(out_d[:], out_sb[:])

    nc.compile()
    return nc


def postprocess(results, batch, b3_rel, W_lin, b_lin, n_graphs):
    """results: list of per-core dicts with 'out_partial' [10, ng]."""
    total = np.zeros_like(np.asarray(results[0]["out_partial"], np.float32))
    for r in results:
        total += np.asarray(r["out_partial"], np.float32)
    cnt = np.bincount(np.asarray(batch, np.int64), minlength=n_graphs).astype(
        np.float32
    )
    cnt = np.maximum(cnt, 1.0)
    W_lin = np.asarray(W_lin, np.float32)
    # per-node bias b3 pools to cnt[g]*b3; after the mean it is just b3.
    logits = (
        total[:, :n_graphs].T / cnt[:, None]
        + np.asarray(b3_rel, np.float32)[None, :] @ W_lin.T
        + np.asarray(b_lin, np.float32)[None, :]
    )
    return logits.astype(np.float32)


# ----------------------------------------------------------------------------
# harness entry point
# ----------------------------------------------------------------------------
from concourse.bass_utils import run_bass_kernel_spmd

_CACHE = {}


def kernel(x, edge_index, batch,
           W1_rel, b1_rel, W1_root,
           W2_rel, b2_rel, W2_root,
           W3_rel, b3_rel, W3_root,
           W_lin, b_lin):
    params = dict(W1_rel=W1_rel, b1_rel=b1_rel, W1_root=W1_root,
                  W2_rel=W2_rel, b2_rel=b2_rel, W2_root=W2_root,
                  W3_rel=W3_rel, b3_rel=b3_rel, W3_root=W3_root,
                  W_lin=W_lin, b_lin=b_lin)
    n_nodes = int(np.asarray(x).shape[0])
    n_graphs = 64
    meta, in_maps = preprocess(x, edge_index, batch, params, n_nodes, n_graphs)
    key = (meta["w_star"], meta["wA_star"], meta["ls"])
    if key not in _CACHE:
        _CACHE[key] = build_nc(meta)
    nc = _CACHE[key]
    res = run_bass_kernel_spmd(nc, in_maps, core_ids=list(range(N_CORES)))
    return postprocess(res.results, batch, b3_rel, W_lin, b_lin, n_graphs)


# revision 9
# speedup vs baseline: 1.0463x; 1.0463x over previous
"""GraphConv GNN kernel for trn2: host preprocessing + bass program builder.

Sharding: nodes (and incident edges, by dst) across 8 cores. Weights
replicated. Structural optimizations over the dma_gather-everywhere baseline
(which was GpSimd-bound on gather descriptor generation):

- Layer 1: the gather of x[src] is a host-side permutation of an input
  tensor, so it is pre-gathered on the host into a sequential stream and
  DMA'd in chunk order (no dma_gather, no Q7 work).
- Layer 2: real dma_gather from the AllGather'd h1 (unavoidable: h1 is
  computed on device and relu is nonlinear). Sources are split into region
  A/B by the source node's local index half, and h1 is exchanged with TWO
  AllGathers (A then B) so the region-A gathers start while the region-B
  AllGather is still in flight.
- Layer 3 is linear and feeds only mean-pooling, so pooling commutes with
  it: pool(agg3)[g] = sum_u C[u,g] h2[u] with C[u,g] = #edges from node u
  into graph g (host-built count matrix), and pool(h2) uses the batch
  one-hot. Both are small dense matmuls over local node chunks; partial
  sums are combined on the host. This removes the layer-3 gather, its
  one-hot streams, and the second h-AllGather entirely.
"""

import sys

sys.path.insert(0, "/opt/trn_rl_repo")

import numpy as np
import ml_dtypes

import concourse.bass as bass
import concourse.bacc as bacc
import concourse.tile as tile
import concourse.mybir as mybir
from concourse import library_config

BF16 = mybir.dt.bfloat16
F32 = mybir.dt.float32
I16 = mybir.dt.int16

N_CORES = 8
F = 128
N_CLASSES = 10

# per-window structure: K_A region-A chunks + K_B region-B chunks of 128 edges
K_A = 6
K_B = 6
EDGES_PER_HALF = K_A * 128  # 768
CHUNKS_PER_WIN = K_A + K_B
CPO = 32  # one-hot stream chunks per DMA op
CPO_G = 64  # gather chunks per dma_gather op (8192 idxs; amortizes the
            # ~17us fixed per-op engine tail observed on HW)
S_FP8 = True  # layer-1 one-hot streams stored fp8, cast to bf16 on DMA


def _wrap_idx(idx_flat):
    """idx i -> partition i%16, col i//16; replicated across the 8 Q7 core
    stripes (16 partitions each)."""
    n = idx_flat.shape[0]
    return np.ascontiguousarray(
        np.tile(idx_flat.reshape(n // 16, 16).T.astype(np.int16), (8, 1))
    )


def preprocess(x, edge_index, batch, params, n_nodes, n_graphs):
    """Build per-core inputs + meta for the SPMD program."""
    assert n_nodes % N_CORES == 0
    npc = n_nodes // N_CORES
    halfn = npc // 2
    src = np.asarray(edge_index[0], np.int64)
    dst = np.asarray(edge_index[1], np.int64)
    batch = np.asarray(batch, np.int64)
    x = np.asarray(x, np.float32)

    # region A: source's LOCAL index (within its owner core) < halfn
    # sort edges by dst once
    order = np.argsort(dst, kind="stable")
    src_s, dst_s = src[order], dst[order]

    # per-core edge ranges
    core_edge_start = np.searchsorted(dst_s, np.arange(0, n_nodes + 1, npc))

    # --- pass 1: greedy windows per core (forced break at halfn) ---
    core_windows = []  # per core: list of (dst_start, dst_end) local
    core_wA = []  # windows covering dst < halfn
    for k in range(N_CORES):
        e0, e1 = core_edge_start[k], core_edge_start[k + 1]
        dl = dst_s[e0:e1] - k * npc
        sl_a = (src_s[e0:e1] % npc) < halfn
        deg_a = np.bincount(dl[sl_a], minlength=npc)
        deg_b = np.bincount(dl[~sl_a], minlength=npc)
        wins = []
        d = 0
        while d < npc:
            start = d
            brk = halfn if d < halfn else npc
            a_c = b_c = 0
            while (
                d < brk
                and d - start < 128
                and a_c + deg_a[d] <= EDGES_PER_HALF
                and b_c + deg_b[d] <= EDGES_PER_HALF
            ):
                a_c += deg_a[d]
                b_c += deg_b[d]
                d += 1
            assert d > start, "single dst exceeds per-window edge budget"
            wins.append((start, d))
        core_windows.append(wins)
        core_wA.append(sum(1 for a, _ in wins if a < halfn))

    wA_star = max(core_wA)
    wB_star = max(len(w) - a for w, a in zip(core_windows, core_wA))
    w_star = wA_star + wB_star
    if w_star % 4:  # keep ls a multiple of 512
        wB_star += 4 - (w_star % 4)
        w_star = wA_star + wB_star
    ls = w_star * 128
    lsA, lsB = wA_star * 128, wB_star * 128
    rowsA, rowsB = N_CORES * lsA, N_CORES * lsB
    assert max(rowsA, rowsB) <= 32768, f"{rowsA=} {rowsB=} exceed int16 idx range"

    # --- slots for every node (A windows at 0..wA*-1, B at wA*..w*-1) ---
    slot = np.full(n_nodes, -1, np.int64)
    for k in range(N_CORES):
        wA_k = core_wA[k]
        for w, (a, b) in enumerate(core_windows[k]):
            w_slab = w if w < wA_k else wA_star + (w - wA_k)
            d_loc = np.arange(a, b)
            slot[k * npc + d_loc] = w_slab * 128 + (d_loc - a)
    assert (slot >= 0).all()
    owner = np.arange(n_nodes) // npc
    wslab = slot // 128
    in_A = wslab < wA_star
    # region-local row index (fm pos s -> (p=s%128, c=s//128); partition-major
    # DRAM -> row = p*W + c)
    row_reg = np.where(
        in_A,
        owner * lsA + (slot % 128) * wA_star + wslab,
        owner * lsB + (slot % 128) * wB_star + (wslab - wA_star),
    )
    # sanity: A-region nodes are exactly the first-half locals
    assert (in_A == ((np.arange(n_nodes) % npc) < halfn)).all()

    x_bf = x.astype(ml_dtypes.bfloat16)

    # --- per-core streams (keys: "a" region A sources, "b" region B) ---
    per_core = []
    for k in range(N_CORES):
        e0, e1 = core_edge_start[k], core_edge_start[k + 1]
        dl = dst_s[e0:e1] - k * npc
        sv = src_s[e0:e1]
        is_a = (sv % npc) < halfn
        idx_a = np.zeros((w_star, EDGES_PER_HALF), np.int64)
        ids_a = np.full((w_star, EDGES_PER_HALF), -1.0, np.float32)
        src_a = np.full((w_star, EDGES_PER_HALF), -1, np.int64)
        idx_b = np.zeros_like(idx_a)
        ids_b = np.full_like(ids_a, -1.0)
        src_b = np.full_like(src_a, -1)
        wbounds = np.searchsorted(
            dl, [a for a, _ in core_windows[k]] + [npc]
        )
        wA_k = core_wA[k]
        for w, (a, b) in enumerate(core_windows[k]):
            w_slab = w if w < wA_k else wA_star + (w - wA_k)
            a_m = is_a[wbounds[w] : wbounds[w + 1]]
            e_dst = dl[wbounds[w] : wbounds[w + 1]]
            e_src = sv[wbounds[w] : wbounds[w + 1]]
            for half, m in ((0, a_m), (1, ~a_m)):
                r = row_reg[e_src[m]]
                cnt = r.shape[0]
                assert cnt <= EDGES_PER_HALF
                tgt_idx = idx_a if half == 0 else idx_b
                tgt_ids = ids_a if half == 0 else ids_b
                tgt_src = src_a if half == 0 else src_b
                tgt_idx[w_slab, :cnt] = r
                tgt_ids[w_slab, :cnt] = (e_dst[m] - a).astype(np.float32)
                tgt_src[w_slab, :cnt] = e_src[m]

        def _onehot(ids_arr, dt):
            nch = ids_arr.size // 128
            ids_r = ids_arr.reshape(nch, 128)
            oh = (ids_r[:, :, None] == np.arange(128, dtype=np.float32)[None, None, :])
            return np.ascontiguousarray(
                oh.transpose(1, 0, 2).reshape(128, nch * 128).astype(dt))

        def _pregather(src_arr):
            # slot j (chunk c=j//128, p=j%128) -> x[src]; layout [128, nch*F]
            flat = src_arr.reshape(-1)
            g = np.zeros((flat.shape[0], F), ml_dtypes.bfloat16)
            v = flat >= 0
            g[v] = x_bf[flat[v]]
            nch = flat.shape[0] // 128
            return np.ascontiguousarray(
                g.reshape(nch, 128, F).transpose(1, 0, 2).reshape(128, nch * F))

        sdt = ml_dtypes.float8_e4m3fn if S_FP8 else ml_dtypes.bfloat16
        per_core.append(
            dict(
                idx_a=_wrap_idx(idx_a.reshape(-1)),
                idx_b=_wrap_idx(idx_b.reshape(-1)),
                s_a=_onehot(ids_a.reshape(-1), ml_dtypes.bfloat16),
                s_b=_onehot(ids_b.reshape(-1), ml_dtypes.bfloat16),
                s8_a=_onehot(ids_a.reshape(-1), sdt),
                s8_b=_onehot(ids_b.reshape(-1), sdt),
                xg_a=_pregather(src_a),
                xg_b=_pregather(src_b),
            )
        )

    # --- per-(src node, graph) edge-count matrix for the pooled layer-3 ---
    gmax = 64
    c_full = np.zeros((n_nodes, gmax), np.float32)
    np.add.at(c_full, (src, batch[dst]), 1.0)

    def _node_major_64(vals_per_node, k):
        """vals [npc, 64] for core k's local nodes -> [128, w_star*64] in
        node-major chunk layout (row p, block c) = node at slot c*128+p."""
        out = np.zeros((ls, gmax), np.float32)
        g = np.arange(k * npc, (k + 1) * npc)
        out[slot[g]] = vals_per_node
        out = out.reshape(w_star, 128, gmax).transpose(1, 0, 2)
        return np.ascontiguousarray(
            out.reshape(128, w_star * gmax).astype(ml_dtypes.bfloat16))

    in_maps = []
    for k in range(N_CORES):
        g = np.arange(k * npc, (k + 1) * npc)
        x_fm = np.zeros((F, ls), ml_dtypes.bfloat16)
        x_fm[:, slot[g]] = x_bf[g].T
        b_vals = np.zeros((npc, gmax), np.float32)
        b_vals[np.arange(npc), batch[g]] = 1.0
        m = dict(
            x_fm=x_fm,
            b_onehot=_node_major_64(b_vals, k),
            c_onehot=_node_major_64(c_full[g], k),
            idx_a=per_core[k]["idx_a"],
            idx_b=per_core[k]["idx_b"],
            s_a=per_core[k]["s_a"],
            s_b=per_core[k]["s_b"],
            xg_a=per_core[k]["xg_a"],
            xg_b=per_core[k]["xg_b"],
            w1relT=np.ascontiguousarray(params["W1_rel"].T.astype(ml_dtypes.bfloat16)),
            w1rootT=np.ascontiguousarray(
                params["W1_root"].T.astype(ml_dtypes.bfloat16)
            ),
            w2relT=np.ascontiguousarray(params["W2_rel"].T.astype(ml_dtypes.bfloat16)),
            w2rootT=np.ascontiguousarray(
                params["W2_root"].T.astype(ml_dtypes.bfloat16)
            ),
            w3relT=np.ascontiguousarray(params["W3_rel"].T.astype(np.float32)),
            w3rootT=np.ascontiguousarray(
                params["W3_root"].T.astype(np.float32)
            ),
            b1=np.ascontiguousarray(params["b1_rel"].astype(np.float32).reshape(F, 1)),
            b2=np.ascontiguousarray(params["b2_rel"].astype(np.float32).reshape(F, 1)),
            wlinT=np.ascontiguousarray(params["W_lin"].T.astype(np.float32)),
        )
        if S_FP8:
            m["s8_a"] = per_core[k]["s8_a"]
            m["s8_b"] = per_core[k]["s8_b"]
        in_maps.append(m)

    meta = dict(w_star=w_star, wA_star=wA_star, wB_star=wB_star,
                ls=ls, lsA=lsA, lsB=lsB, rowsA=rowsA, rowsB=rowsB,
                n_graphs=n_graphs)
    return meta, in_maps


def build_nc(meta, n_graphs_pad=64):
    w_star = meta["w_star"]
    wA_star, wB_star = meta["wA_star"], meta["wB_star"]
    ls, lsA, lsB = meta["ls"], meta["lsA"], meta["lsB"]
    rowsA, rowsB = meta["rowsA"], meta["rowsB"]
    sl_len = w_star * EDGES_PER_HALF  # idxs per region stream
    n_chunks = sl_len // 128
    dw = ls // 512  # dense windows
    ng = n_graphs_pad
    FP8 = mybir.dt.float8e4

    nc = bacc.Bacc(
        "TRN2", target_bir_lowering=False, debug=False, num_devices=N_CORES
    )

    # --- I/O ---
    x_fm_d = nc.dram_tensor("x_fm", [F, ls], BF16, kind="ExternalInput")
    bone_d = nc.dram_tensor("b_onehot", [128, w_star * 64], BF16, kind="ExternalInput")
    cone_d = nc.dram_tensor("c_onehot", [128, w_star * 64], BF16, kind="ExternalInput")
    idx_d = {
        "a": nc.dram_tensor("idx_a", [128, sl_len // 16], I16, kind="ExternalInput"),
        "b": nc.dram_tensor("idx_b", [128, sl_len // 16], I16, kind="ExternalInput"),
    }
    s_d = {
        "a": nc.dram_tensor("s_a", [128, n_chunks * 128], BF16, kind="ExternalInput"),
        "b": nc.dram_tensor("s_b", [128, n_chunks * 128], BF16, kind="ExternalInput"),
    }
    if S_FP8:
        s8_d = {
            "a": nc.dram_tensor("s8_a", [128, n_chunks * 128], FP8, kind="ExternalInput"),
            "b": nc.dram_tensor("s8_b", [128, n_chunks * 128], FP8, kind="ExternalInput"),
        }
    xg_d = {
        "a": nc.dram_tensor("xg_a", [128, n_chunks * F], BF16, kind="ExternalInput"),
        "b": nc.dram_tensor("xg_b", [128, n_chunks * F], BF16, kind="ExternalInput"),
    }
    w_d = {}
    for l in (1, 2):
        for p in ("rel", "root"):
            w_d[l, p] = nc.dram_tensor(f"w{l}{p}T", [F, F], BF16, kind="ExternalInput")
    w3_d = {
        "rel": nc.dram_tensor("w3relT", [F, F], F32, kind="ExternalInput"),
        "root": nc.dram_tensor("w3rootT", [F, F], F32, kind="ExternalInput"),
    }
    b_d = {l: nc.dram_tensor(f"b{l}", [F, 1], F32, kind="ExternalInput") for l in (1, 2)}
    wlin_d = nc.dram_tensor("wlinT", [F, N_CLASSES], F32, kind="ExternalInput")
    out_d = nc.dram_tensor("out_partial", [N_CLASSES, ng], F32, kind="ExternalOutput")

    relu = mybir.ActivationFunctionType.Relu
    copy_f = mybir.ActivationFunctionType.Copy

    n_ops = (n_chunks + CPO - 1) // CPO
    n_ops_g = (n_chunks + CPO_G - 1) // CPO_G

    with tile.TileContext(nc) as tc:
        with (
            tc.tile_pool(name="const", bufs=1) as constp,
            tc.tile_pool(name="state", bufs=1) as statep,
            tc.tile_pool(name="gpool", bufs=2) as gpool,
            tc.tile_pool(name="spool", bufs=8) as spool,
            tc.tile_pool(name="psa", bufs=2, space="PSUM") as psa,
            tc.tile_pool(name="psd", bufs=2, space="PSUM") as psd,
            tc.tile_pool(name="psp", bufs=1, space="PSUM") as psp,
            tc.tile_pool(name="dram", bufs=1, space="DRAM") as dramp,
        ):
            nc.gpsimd.load_library(library_config.mlp)

            # ---- load constants ----
            bone_t = constp.tile([128, w_star * 64], BF16)
            nc.sync.dma_start(bone_t[:], bone_d[:])
            cone_t = constp.tile([128, w_star * 64], BF16)
            nc.sync.dma_start(cone_t[:], cone_d[:])
            idx_t = {}
            for h in ("a", "b"):
                it = constp.tile([128, sl_len // 16], I16, name=f"idx_{h}")
                nc.sync.dma_start(it[:], idx_d[h][:])
                idx_t[h] = it
            w_t = {}
            for key, d in w_d.items():
                wt = constp.tile([F, F], BF16, name=f"w_{key[0]}_{key[1]}")
                nc.sync.dma_start(wt[:], d[:])
                w_t[key] = wt
            w3_t = {}
            for p, d in w3_d.items():
                wt = constp.tile([F, F], F32, name=f"w3_{p}")
                nc.sync.dma_start(wt[:], d[:])
                w3_t[p] = wt
            b_t = {}
            for l, d in b_d.items():
                bt = constp.tile([F, 1], F32, name=f"b_{l}")
                nc.sync.dma_start(bt[:], d[:])
                b_t[l] = bt
            wlin_t = constp.tile([F, N_CLASSES], F32)
            nc.sync.dma_start(wlin_t[:], wlin_d[:])

            x_fm_t = statep.tile([F, ls], BF16, tag="h0")
            nc.sync.dma_start(x_fm_t[:], x_fm_d[:])

            # ---- layers 1 (pre-gathered stream) and 2 (dma_gather) ----
            h_fm = x_fm_t
            hf = {}  # layer-2 gather sources (set after AllGathers)
            for layer in (1, 2):
                g_tiles = {"a": [], "b": []}
                s_tiles = {"a": [], "b": []}
                # region A ops first, then region B (B AllGather overlaps A ops)
                for h in ("a", "b"):
                    for o in range(n_ops):
                        c0 = o * CPO
                        c1 = min(n_chunks, c0 + CPO)
                        nch = c1 - c0
                        st_ = spool.tile(
                            [128, nch * 128],
                            BF16,
                            name=f"sg_{layer}_{h}_{o}",
                            tag=f"sg_{h}",
                            bufs=2,
                        )
                        if layer == 1 and S_FP8:
                            nc.gpsimd.dma_start(
                                st_[:], s8_d[h][:, c0 * 128 : c1 * 128]
                            )
                        else:
                            nc.sync.dma_start(
                                st_[:], s_d[h][:, c0 * 128 : c1 * 128]
                            )
                        s_tiles[h].append(st_)
                    for o in range(n_ops_g):
                        c0 = o * CPO_G
                        c1 = min(n_chunks, c0 + CPO_G)
                        nch = c1 - c0
                        gt = gpool.tile(
                            [128, nch, F],
                            BF16,
                            name=f"g_{layer}_{h}_{o}",
                            tag=f"g_{h}",
                            padded_shape=[128, CPO_G, F],
                        )
                        if layer == 1:
                            nc.sync.dma_start(
                                gt[:], xg_d[h][:, c0 * F : c1 * F]
                            )
                        else:
                            nidx = nch * 128
                            nc.gpsimd.dma_gather(
                                gt[:],
                                hf[h][:],
                                idx_t[h][:, c0 * 8 : c1 * 8],
                                nidx,
                                nidx,
                                F,
                                single_packet=False,
                            )
                        g_tiles[h].append(gt)

                # aggregation windows (agg1 is dead before agg2 is written, so
                # the two layers share one SBUF buffer via the common tag)
                agg_fm = statep.tile([F, ls], BF16, tag="agg", name=f"agg_{layer}")
                for w in range(w_star):
                    ps = psa.tile([128, 128], F32, name=f"psagg_{layer}_{w}", tag="psagg")
                    for j in range(CHUNKS_PER_WIN):
                        h = "a" if j < K_A else "b"
                        cc = w * K_A + (j % K_A)
                        o, sl_ = cc // CPO, cc % CPO
                        og, slg = cc // CPO_G, cc % CPO_G
                        nc.tensor.matmul(
                            ps[:],
                            g_tiles[h][og][:, slg, :],
                            s_tiles[h][o][:, sl_ * 128 : (sl_ + 1) * 128],
                            start=(j == 0),
                            stop=(j == CHUNKS_PER_WIN - 1),
                        )
                    nc.scalar.activation(
                        agg_fm[:, w * 128 : (w + 1) * 128], ps[:], copy_f
                    )

                # dense
                h_next = statep.tile([F, ls], BF16, tag=f"h{layer}", name=f"h_{layer}")
                for d in range(dw):
                    ps = psd.tile([128, 512], F32, name=f"psd_{layer}_{d}", tag="psd")
                    sl2 = slice(d * 512, (d + 1) * 512)
                    nc.tensor.matmul(
                        ps[:], w_t[layer, "rel"][:], agg_fm[:, sl2], start=True, stop=False
                    )
                    nc.tensor.matmul(
                        ps[:], w_t[layer, "root"][:], h_fm[:, sl2], start=False, stop=True
                    )
                    nc.scalar.activation(
                        h_next[:, sl2],
                        ps[:],
                        relu,
                        bias=b_t[layer][:],
                    )

                if layer == 1:
                    # share h1 for the layer-2 gather: region A first so AG_A
                    # lands while AG_B is still in flight.
                    h_nm = statep.tile(
                        [128, w_star, F], BF16, tag="h1nm", name="hnm_1"
                    )
                    for h, w0, w1, lsR, rowsR in (
                        ("a", 0, wA_star, lsA, rowsA),
                        ("b", wA_star, w_star, lsB, rowsB),
                    ):
                        nc.sync.dma_start_transpose(
                            h_nm[:, w0:w1, :], h_next[:, w0 * 128 : w1 * 128]
                        )
                        ag_in = dramp.tile(
                            [128, lsR], BF16, name=f"agin_{h}", tag=f"agin{h}"
                        )
                        hf_r = dramp.tile(
                            [rowsR, F],
                            BF16,
                            name=f"hf_{h}",
                            tag=f"hf{h}",
                            addr_space="Shared",
                        )
                        nc.sync.dma_start(ag_in[:], h_nm[:, w0:w1, :])
                        nc.gpsimd.collective_compute(
                            "AllGather",
                            mybir.AluOpType.bypass,
                            replica_groups=[list(range(N_CORES))],
                            ins=[ag_in[:]],
                            outs=[hf_r[:]],
                        )
                        hf[h] = hf_r
                    h_fm = h_next
                else:
                    h2_fm = h_next

            # ---- pooled layer 3: pool(agg3) = C^T h2, pool(h2) = B^T h2 ----
            h2_nm = statep.tile([128, w_star, F], BF16, tag="h2nm", name="hnm_2")
            nc.sync.dma_start_transpose(h2_nm[:], h2_fm[:])

            ps_pa = psp.tile([128, ng], F32, tag="pspa")
            ps_ph = psp.tile([128, ng], F32, tag="psph")
            for c in range(w_star):
                nc.tensor.matmul(
                    ps_pa[:],
                    h2_nm[:, c, :],
                    cone_t[:, c * ng : (c + 1) * ng],
                    start=(c == 0),
                    stop=(c == w_star - 1),
                )
            for c in range(w_star):
                nc.tensor.matmul(
                    ps_ph[:],
                    h2_nm[:, c, :],
                    bone_t[:, c * ng : (c + 1) * ng],
                    start=(c == 0),
                    stop=(c == w_star - 1),
                )
            pa_sb = statep.tile([128, ng], F32, tag="pasb")
            ph_sb = statep.tile([128, ng], F32, tag="phsb")
            nc.scalar.activation(pa_sb[:], ps_pa[:], copy_f)
            nc.scalar.activation(ph_sb[:], ps_ph[:], copy_f)

            # z3p = W3_rel @ pool(agg3) + W3_root @ pool(h2)   [128, ng] f32
            ps_z = psp.tile([128, ng], F32, tag="psz")
            nc.tensor.matmul(ps_z[:], w3_t["rel"][:], pa_sb[:], start=True, stop=False)
            nc.tensor.matmul(ps_z[:], w3_t["root"][:], ph_sb[:], start=False, stop=True)
            z_sb = statep.tile([128, ng], F32, tag="zsb")
            nc.vector.tensor_copy(z_sb[:], ps_z[:])

            ps_head = psp.tile([N_CLASSES, ng], F32, tag="pshead")
            nc.tensor.matmul(ps_head[:], wlin_t[:], z_sb[:])
            out_sb = statep.tile([N_CLASSES, ng], F32, tag="outsb")
            nc.vector.tensor_copy(out_sb[:], ps_head[:])
            nc.sync.dma_start(out_d[:], out_sb[:])

    nc.compile()
    return nc


def postprocess(results, batch, b3_rel, W_lin, b_lin, n_graphs):
    """results: list of per-core dicts with 'out_partial' [10, ng]."""
    total = np.zeros_like(np.asarray(results[0]["out_partial"], np.float32))
    for r in results:
        total += np.asarray(r["out_partial"], np.float32)
    cnt = np.bincount(np.asarray(batch, np.int64), minlength=n_graphs).astype(
        np.float32
    )
    cnt = np.maximum(cnt, 1.0)
    W_lin = np.asarray(W_lin, np.float32)
    # per-node bias b3 pools to cnt[g]*b3; after the mean it is just b3.
    logits = (
        total[:, :n_graphs].T / cnt[:, None]
        + np.asarray(b3_rel, np.float32)[None, :] @ W_lin.T
        + np.asarray(b_lin, np.float32)[None, :]
    )
    return logits.astype(np.float32)


# ----------------------------------------------------------------------------
# harness entry point
# ----------------------------------------------------------------------------
from concourse.bass_utils import run_bass_kernel_spmd

_CACHE = {}


def kernel(x, edge_index, batch,
           W1_rel, b1_rel, W1_root,
           W2_rel, b2_rel, W2_root,
           W3_rel, b3_rel, W3_root,
           W_lin, b_lin):
    params = dict(W1_rel=W1_rel, b1_rel=b1_rel, W1_root=W1_root,
                  W2_rel=W2_rel, b2_rel=b2_rel, W2_root=W2_root,
                  W3_rel=W3_rel, b3_rel=b3_rel, W3_root=W3_root,
                  W_lin=W_lin, b_lin=b_lin)
    n_nodes = int(np.asarray(x).shape[0])
    n_graphs = 64
    meta, in_maps = preprocess(x, edge_index, batch, params, n_nodes, n_graphs)
    key = (meta["w_star"], meta["wA_star"], meta["ls"])
    if key not in _CACHE:
        _CACHE[key] = build_nc(meta)
    nc = _CACHE[key]
    res = run_bass_kernel_spmd(nc, in_maps, core_ids=list(range(N_CORES)))
    return postprocess(res.results, batch, b3_rel, W_lin, b_lin, n_graphs)
